# revision 8
# baseline (speedup 1.0000x reference)
"""GAT layer (gnn_message_passing) on 8 Trainium2 NeuronCores.

Strategy v2 (no collectives, full-z recompute per core, dst-major slots):
  - Every core computes the FULL node feature table z = h @ W.T (bf16) from a
    replicated bf16 copy of h^T: redundant compute is far cheaper than the
    AllGather it replaces.  Table rows are 512B: [z bf16[128] | zs_hi | zs_lo |
    garbage pad] -- zs is carried at ~f32 precision as a bf16 hi/lo pair, and
    the pad is never read so it is never initialized.
  - Host relabels each core's owned nodes by descending half-0 in-degree
    ("slot0" order) and ships h^T in that order, so half-0's segment layout
    needs no on-device permutation at all.  Half-1 keeps its own degree sort
    plus one small dma_scatter_add to un-permute its partial sums.
  - Edges are bucketed by dst owner (each core holds the complete in-edge set
    of its nodes -> fully local segment softmax, no max subtraction needed),
    then split by src half so gather indices fit int16.  dma_gather fetches
    src rows straight into the dst-major slot layout; the weighted segment sum
    runs on the TensorEngine as identity-lhsT matmul chains in PSUM.
  - The table is stored per half, rows interleaved partition-major
    (row = p*196 + tile) so table writes are large contiguous DMA runs.
  - zd (and an own-node zs) comes from a tiny per-core matmul against the
    core's own h^T slice, avoiding core-dependent addressing in the shared
    SPMD program.
  - Zero-in-degree nodes get a fake self-edge on the host so out == z for
    them (to bf16 accuracy).
"""

import numpy as np
import ml_dtypes

import concourse.bass as bass
import concourse.mybir as mybir
import concourse.tile as tile
from concourse import bacc
from concourse import library_config
from concourse.bass import ts
from concourse.bass_utils import run_bass_kernel_spmd

F32 = mybir.dt.float32
BF16 = mybir.dt.bfloat16
I16 = mybir.dt.int16

NC = 8          # cores
P = 128         # partitions
IN_DIM = 256
OUT_DIM = 128
KCH = IN_DIM // P       # 2 k-chunks for the feature matmul
ROW_ELEMS = 256         # bf16 elems per table row (512 B)
ACC_STRIDE = 320        # f32 elems per acc row (1280 B, multiple of 256 B)
SCAT_ELEMS = 129        # f32 elems scattered per slot ([agg128 | den])
ZDP_STRIDE = 64         # f32 stride of the zd permute buffer (256 B)
CHUNK_COLS = 48         # max gather columns per chunk
STAGE_TILES = 12        # tiles per table-write staging buffer
PSG = 3                 # tiles per PSUM group (3*130 f32 fits one 2KB bank)


class Cfg:
    def __init__(self, n_nodes, n_edges):
        assert n_nodes % NC == 0
        self.N = n_nodes
        self.E = n_edges
        self.NPC = n_nodes // NC
        self.NPAD = ((self.NPC + P - 1) // P) * P
        self.NT = self.NPAD // P
        self.NTOT = NC * self.NPAD          # global relabeled node count
        self.HALF_ROWS = self.NTOT // 2     # table rows per half
        self.HALF_T = self.HALF_ROWS // P   # table tiles per half
        assert self.HALF_ROWS < 32768


def _wrap16(flat, dtype=np.int16):
    """flat[i] -> [128, len/16] with flat[i] at [i%16, i//16], replicated x8."""
    n = flat.shape[0]
    assert n % 16 == 0
    w = flat.reshape(n // 16, 16).T.astype(dtype)  # [16, n/16]
    return np.tile(w, (8, 1))


def _copy(eng, out, in_):
    if hasattr(eng, "tensor_copy"):
        eng.tensor_copy(out, in_)
    else:
        eng.copy(out, in_)


def _mk_chunks(W, cap):
    """Runs of equal width, capped at cap columns -> [(t0, ntiles, w)]."""
    chunks = []
    t = 0
    ntp = len(W)
    while t < ntp:
        w = int(W[t])
        nt = 1
        while t + nt < ntp and int(W[t + nt]) == w and (nt + 1) * w <= cap:
            nt += 1
        chunks.append((t, nt, w))
        t += nt
    return chunks


def host_prep(cfg, src, dst):
    """Relabel nodes, build slot layouts + per-core index/mask arrays."""
    N, NPC, NPAD, NT = cfg.N, cfg.NPC, cfg.NPAD, cfg.NT
    HALF_T = cfg.HALF_T
    src = np.asarray(src, np.int64).copy()
    dst = np.asarray(dst, np.int64).copy()

    # fake self-edges for isolated (zero in-degree) nodes -> out == z
    deg_tot = np.bincount(dst, minlength=N)
    iso = np.nonzero(deg_tot == 0)[0]
    if iso.size:
        src = np.concatenate([src, iso])
        dst = np.concatenate([dst, iso])

    owner = dst // NPC
    halves = (src // NPC >= NC // 2).astype(np.int64)

    # --- per-core relabel by half-0 degree; half-1 gets its own sort ---
    order0 = np.zeros((NC, NPAD), np.int64)   # slot0 -> orig local
    sL0 = np.zeros((NC, NPAD), np.int64)      # orig local -> slot0
    order1 = np.zeros((NC, NPAD), np.int64)   # slot1 -> relabeled local
    sL1 = np.zeros((NC, NPAD), np.int64)      # relabeled local -> slot1
    deg0s = np.zeros((NC, NPAD), np.int64)
    deg1s = np.zeros((NC, NPAD), np.int64)
    for c in range(NC):
        m = owner == c
        d_local = dst[m] - c * NPC
        h = halves[m]
        deg0 = np.bincount(d_local[h == 0], minlength=NPAD)
        deg1 = np.bincount(d_local[h == 1], minlength=NPAD)
        o0 = np.argsort(-deg0, kind="stable")
        s0 = np.empty(NPAD, np.int64)
        s0[o0] = np.arange(NPAD)
        deg1r = deg1[o0]                       # half-1 degree in relabeled order
        o1 = np.argsort(-deg1r, kind="stable")
        s1 = np.empty(NPAD, np.int64)
        s1[o1] = np.arange(NPAD)
        order0[c], sL0[c], order1[c], sL1[c] = o0, s0, o1, s1
        deg0s[c] = deg0[o0]                    # sorted desc
        deg1s[c] = deg1r[o1]                   # sorted desc

    # common tile widths per half (max over cores; per-core arrays sorted)
    W0 = deg0s.reshape(NC, NT, P).max(axis=2).max(axis=0)
    W1 = deg1s.reshape(NC, NT, P).max(axis=2).max(axis=0)
    NTp0 = int(np.nonzero(W0 > 0)[0][-1]) + 1 if (W0 > 0).any() else 0
    NTp1 = int(np.nonzero(W1 > 0)[0][-1]) + 1 if (W1 > 0).any() else 0
    W0, W1 = W0[:NTp0], W1[:NTp1]
    colstart0 = np.concatenate([[0], np.cumsum(W0)]).astype(np.int64)
    colstart1 = np.concatenate([[0], np.cumsum(W1)]).astype(np.int64)
    C0, C1 = int(colstart0[-1]), int(colstart1[-1])
    chunks0 = _mk_chunks(W0, CHUNK_COLS)
    chunks1 = _mk_chunks(W1, CHUNK_COLS)

    # global relabeled id + interleaved table row of each ORIGINAL node
    g_of_orig = np.empty(N, np.int64)
    for c in range(NC):
        loc = np.arange(NPC)
        g_of_orig[c * NPC:(c + 1) * NPC] = c * NPAD + sL0[c][loc]
    q = g_of_orig % cfg.HALF_ROWS
    row_of_orig = (q % P) * HALF_T + (q // P)
    assert row_of_orig.max() < 32768

    src_row = row_of_orig[src]

    data = {}
    for c in range(NC):
        per = {}
        for h in (0, 1):
            m = (owner == c) & (halves == h)
            es = src_row[m]
            d_rel = sL0[c][dst[m] - c * NPC]          # relabeled local dst
            slot = d_rel if h == 0 else sL1[c][d_rel]
            W, colstart, NTp, C = (
                (W0, colstart0, NTp0, C0) if h == 0 else (W1, colstart1, NTp1, C1)
            )
            o = np.argsort(slot, kind="stable")
            slot_s, es_s = slot[o], es[o]
            counts = np.bincount(slot_s, minlength=NPAD)
            starts = np.concatenate([[0], np.cumsum(counts)])[:-1]
            rank = np.arange(slot_s.size) - starts[slot_s]
            tile_s = slot_s // P
            part_s = slot_s % P
            assert (tile_s < NTp).all() and (rank < W[tile_s]).all()
            cglob = colstart[tile_s] + rank
            pos = cglob * P + part_s
            flat_idx = np.zeros(C * P, np.int16)
            flat_idx[pos] = es_s.astype(np.int16)
            mask = np.zeros((P, C), ml_dtypes.bfloat16)
            mask[part_s, cglob] = 1.0
            per[f"gidx{h}"] = _wrap16(flat_idx)
            per[f"gmask{h}"] = mask
        per["mscat1"] = _wrap16(order1[c][: max(NTp1, 1) * P].astype(np.int16))
        per["zdscat1"] = _wrap16(sL1[c].astype(np.int16))
        data[c] = per

    struct = dict(
        W0=W0, W1=W1, NTp0=NTp0, NTp1=NTp1,
        colstart0=colstart0, colstart1=colstart1, C0=C0, C1=C1,
        chunks0=chunks0, chunks1=chunks1,
        order0=order0, iso=iso,
    )
    return struct, data


def build_program(cfg, struct):
    NPAD, NT, NTOT, HALF_T = cfg.NPAD, cfg.NT, cfg.NTOT, cfg.HALF_T
    HALF_ROWS = cfg.HALF_ROWS
    NTp0, NTp1 = struct["NTp0"], struct["NTp1"]
    C0, C1 = struct["C0"], struct["C1"]

    nc = bacc.Bacc(
        "TRN2", target_bir_lowering=False, debug=False, num_devices=NC
    )

    # I/O
    hT = nc.dram_tensor("hT", [IN_DIM, NTOT], BF16, kind="ExternalInput").ap()
    hTown = nc.dram_tensor("hTown", [IN_DIM, NPAD], BF16, kind="ExternalInput").ap()
    W_aug = nc.dram_tensor("W_aug", [IN_DIM, 130], BF16, kind="ExternalInput").ap()
    ident_in = nc.dram_tensor("ident", [P, P], BF16, kind="ExternalInput").ap()
    gidx0_in = nc.dram_tensor("gidx0", [P, C0 * 8], I16, kind="ExternalInput").ap()
    gmask0_in = nc.dram_tensor("gmask0", [P, C0], BF16, kind="ExternalInput").ap()
    gidx1_in = nc.dram_tensor("gidx1", [P, C1 * 8], I16, kind="ExternalInput").ap()
    gmask1_in = nc.dram_tensor("gmask1", [P, C1], BF16, kind="ExternalInput").ap()
    mscat1_in = nc.dram_tensor(
        "mscat1", [P, max(NTp1, 1) * 8], I16, kind="ExternalInput"
    ).ap()
    zdscat1_in = nc.dram_tensor(
        "zdscat1", [P, NPAD // 16], I16, kind="ExternalInput"
    ).ap()

    out = nc.dram_tensor("out", [NPAD, OUT_DIM], F32, kind="ExternalOutput").ap()
    acc = nc.dram_tensor("acc", [NPAD, ACC_STRIDE], F32, kind="ExternalOutput").ap()
    zdperm1 = nc.dram_tensor(
        "zdperm1", [NPAD, ZDP_STRIDE], F32, kind="ExternalOutput"
    ).ap()

    tables = [
        nc.dram_tensor("tableL", [HALF_ROWS, ROW_ELEMS], BF16, kind="Internal").ap(),
        nc.dram_tensor("tableH", [HALF_ROWS, ROW_ELEMS], BF16, kind="Internal").ap(),
    ]

    nc.gpsimd.load_library(library_config.mlp)

    with tile.TileContext(nc) as tc:
        with tc.tile_pool(name="const", bufs=1) as constp:
            ident = constp.tile([P, P], BF16)
            nc.sync.dma_start(ident, ident_in)
            wsb = constp.tile([P, KCH, 130], BF16)
            nc.sync.dma_start(wsb, W_aug.rearrange("(ko ki) m -> ki ko m", ki=P))
            zd_sb = constp.tile([P, NT, 1], F32)    # own zd in slot0 order

            # ---------------- own-node zs/zd pass ----------------
            with (
                tc.tile_pool(name="own", bufs=1) as ownp,
                tc.tile_pool(name="ownps", bufs=2, space="PSUM") as ownps,
            ):
                hob = ownp.tile([P, KCH, NPAD], BF16)
                nc.sync.dma_start(
                    hob, hTown.rearrange("(ko ki) n -> ki ko n", ki=P)
                )
                for g in range(0, NT, 7):
                    gn = min(7, NT - g)
                    ps = ownps.tile([P, 7, 2], F32, tag="ownps")
                    for i in range(gn):
                        for k in range(KCH):
                            nc.tensor.matmul(
                                ps[:, i, :],
                                lhsT=hob[:, k, ts(g + i, P)],
                                rhs=wsb[:, k, 128:130],
                                start=(k == 0),
                                stop=(k == KCH - 1),
                            )
                    nc.vector.tensor_copy(zd_sb[:, g:g + gn, :], ps[:, 0:gn, 1:2])

            # ---------------- phase 1: full z table ----------------
            with (
                tc.tile_pool(name="ph1h", bufs=2) as ph1h,
                tc.tile_pool(name="ph1s", bufs=2) as ph1s,
                tc.tile_pool(name="ph1ps", bufs=2, space="PSUM") as ph1ps,
            ):
                cast_engines = [nc.vector, nc.scalar]
                ce = 0
                for hb in (0, 1):
                    tview = tables[hb].rearrange("(p t) d -> p t d", p=P)
                    for blk0 in range(0, HALF_T, STAGE_TILES):
                        nblk = min(STAGE_TILES, HALF_T - blk0)
                        col0 = hb * HALF_ROWS + blk0 * P
                        hsb = ph1h.tile([P, KCH, STAGE_TILES * P], BF16, tag="hsb")
                        nc.sync.dma_start(
                            hsb[:, :, 0:nblk * P],
                            hT.rearrange("(ko ki) n -> ki ko n", ki=P)[
                                :, :, col0:col0 + nblk * P
                            ],
                        )
                        stage = ph1s.tile([P, STAGE_TILES, ROW_ELEMS], BF16, tag="st")
                        for g0 in range(0, nblk, PSG):
                            gn = min(PSG, nblk - g0)
                            ps = ph1ps.tile([P, PSG, 130], F32, tag="ph1ps")
                            for i in range(gn):
                                for k in range(KCH):
                                    nc.tensor.matmul(
                                        ps[:, i, :],
                                        lhsT=hsb[:, k, ts(g0 + i, P)],
                                        rhs=wsb[:, k, :],
                                        start=(k == 0),
                                        stop=(k == KCH - 1),
                                    )
                            eng = cast_engines[ce % 2]
                            ce += 1
                            # z + zs_hi in one cast; zs_lo = zs - zs_hi
                            _copy(eng, stage[:, g0:g0 + gn, 0:129], ps[:, 0:gn, 0:129])
                            nc.vector.tensor_tensor(
                                stage[:, g0:g0 + gn, 129],
                                ps[:, 0:gn, 128],
                                stage[:, g0:g0 + gn, 128],
                                mybir.AluOpType.subtract,
                            )
                        nc.sync.dma_start(
                            tview[:, blk0:blk0 + nblk, :], stage[:, 0:nblk, :]
                        )

            # ---------------- phase 2: edges ----------------
            aggs = []
            with (
                tc.tile_pool(name="meta", bufs=1) as metap,
                tc.tile_pool(name="aggp", bufs=1) as aggp,
                tc.tile_pool(name="gbuf", bufs=2) as gbuf,
                tc.tile_pool(name="ebuf", bufs=3) as ebuf,
                tc.tile_pool(name="exzb", bufs=2) as exzb,
                tc.tile_pool(name="ps2", bufs=6, space="PSUM") as ps2,
            ):
                gidx_sb = []
                gmask_sb = []
                for h, (gi, gm, C) in enumerate(
                    [(gidx0_in, gmask0_in, C0), (gidx1_in, gmask1_in, C1)]
                ):
                    g = metap.tile([P, C * 8], I16, tag=f"gidx{h}")
                    nc.sync.dma_start(g, gi)
                    gidx_sb.append(g)
                    m = metap.tile([P, C], BF16, tag=f"gmask{h}")
                    nc.sync.dma_start(m, gm)
                    gmask_sb.append(m)

                # zd for half 1: permute via scatter + reload
                zdsc = metap.tile([P, NPAD // 16], I16, tag="zdsc")
                nc.sync.dma_start(zdsc, zdscat1_in)
                nc.gpsimd.dma_scatter_add(
                    out_ap=zdperm1[:, 0:1],
                    in_ap=zd_sb,
                    idxs_ap=zdsc,
                    num_idxs=NPAD,
                    num_idxs_reg=NPAD,
                    elem_size=1,
                    elem_step=ZDP_STRIDE,
                    single_packet=False,
                )
                zdp1 = metap.tile([P, NT, 1], F32, tag="zdp1")
                nc.sync.dma_start(
                    zdp1, zdperm1.rearrange("(t p) d -> p t d", p=P)[:, :, 0:1]
                )

                for h in (0, 1):
                    NTp = NTp0 if h == 0 else NTp1
                    chunks = struct["chunks0"] if h == 0 else struct["chunks1"]
                    colstart = struct["colstart0"] if h == 0 else struct["colstart1"]
                    agg = aggp.tile([P, NT, SCAT_ELEMS], F32, tag=f"agg{h}")
                    aggs.append(agg)
                    if NTp < NT:
                        nc.vector.memset(agg[:, NTp:NT, :], 0)
                    zdv = zd_sb if h == 0 else zdp1
                    cp = 0
                    for (t0, ntc, w) in chunks:
                        cc = ntc * w
                        c0 = int(colstart[t0])
                        G = gbuf.tile([P, CHUNK_COLS, ROW_ELEMS], BF16, tag="G")
                        nc.gpsimd.dma_gather(
                            out_ap=G[:, 0:cc, :],
                            in_ap=tables[h],
                            idxs_ap=gidx_sb[h][:, c0 * 8:(c0 + cc) * 8],
                            num_idxs=cc * P,
                            num_idxs_reg=cc * P,
                            elem_size=ROW_ELEMS,
                            single_packet=cc * P <= 1024,
                        )
                        zsc = ebuf.tile([P, CHUNK_COLS], F32, tag="zsc")
                        nc.vector.tensor_tensor(
                            zsc[:, 0:cc], G[:, 0:cc, 128], G[:, 0:cc, 129],
                            mybir.AluOpType.add,
                        )
                        score = ebuf.tile([P, CHUNK_COLS], F32, tag="score")
                        sc = score[:, 0:cc].rearrange("p (t w) -> p t w", w=w)
                        nc.vector.tensor_tensor(
                            sc,
                            zsc[:, 0:cc].rearrange("p (t w) -> p t w", w=w),
                            zdv[:, t0:t0 + ntc, :].to_broadcast([P, ntc, w]),
                            mybir.AluOpType.add,
                        )
                        nc.vector.scalar_tensor_tensor(
                            score[:, 0:cc], score[:, 0:cc], 0.01, score[:, 0:cc],
                            op0=mybir.AluOpType.mult, op1=mybir.AluOpType.max,
                        )
                        exf = ebuf.tile([P, CHUNK_COLS], BF16, tag="exf")
                        nc.scalar.activation(
                            exf[:, 0:cc], score[:, 0:cc],
                            mybir.ActivationFunctionType.Exp,
                        )
                        exm = ebuf.tile([P, CHUNK_COLS], BF16, tag="exm")
                        nc.vector.tensor_tensor(
                            exm[:, 0:cc], exf[:, 0:cc],
                            gmask_sb[h][:, c0:c0 + cc],
                            mybir.AluOpType.mult,
                        )
                        nc.vector.tensor_reduce(
                            agg[:, t0:t0 + ntc, 128],
                            exm[:, 0:cc].rearrange("p (t w) -> p t w", w=w),
                            mybir.AxisListType.X,
                            mybir.AluOpType.add,
                        )
                        exz = exzb.tile([P, CHUNK_COLS, OUT_DIM], BF16, tag="exz")
                        nc.vector.tensor_tensor(
                            exz[:, 0:cc, :],
                            G[:, 0:cc, 0:OUT_DIM],
                            exm[:, 0:cc, None].to_broadcast([P, cc, OUT_DIM]),
                            mybir.AluOpType.mult,
                        )
                        for ti in range(ntc):
                            ps = ps2.tile([P, OUT_DIM], F32, tag="aggps")
                            for r in range(w):
                                nc.tensor.matmul(
                                    ps,
                                    lhsT=ident,
                                    rhs=exz[:, ti * w + r, :],
                                    start=(r == 0),
                                    stop=(r == w - 1),
                                )
                            eng = nc.scalar if cp % 2 == 0 else nc.vector
                            cp += 1
                            _copy(eng, agg[:, t0 + ti, 0:OUT_DIM], ps)

                # half-1 un-permute: scatter agg1 (slot1 order) into acc rows
                msc = metap.tile([P, max(NTp1, 1) * 8], I16, tag="mscat1")
                nc.sync.dma_start(msc, mscat1_in)
                nc.gpsimd.dma_scatter_add(
                    out_ap=acc[:, 0:SCAT_ELEMS],
                    in_ap=aggs[1][:, 0:max(NTp1, 1), :],
                    idxs_ap=msc,
                    num_idxs=max(NTp1, 1) * P,
                    num_idxs_reg=max(NTp1, 1) * P,
                    elem_size=SCAT_ELEMS,
                    elem_step=ACC_STRIDE,
                    single_packet=False,
                )

                # ---------------- phase 3: combine + divide ----------------
                with tc.tile_pool(name="fin", bufs=1) as finp:
                    acc1 = finp.tile([P, NT, SCAT_ELEMS], F32)
                    nc.sync.dma_start(
                        acc1,
                        acc.rearrange("(t p) d -> p t d", p=P)[:, :, 0:SCAT_ELEMS],
                    )
                    accs = finp.tile([P, NT, SCAT_ELEMS], F32)
                    nc.vector.tensor_tensor(
                        accs, aggs[0], acc1, mybir.AluOpType.add
                    )
                    den = finp.tile([P, NT], F32)
                    nc.vector.tensor_scalar(
                        den, accs[:, :, 128], 1e-30, None, mybir.AluOpType.max
                    )
                    rec = finp.tile([P, NT], F32)
                    nc.vector.reciprocal(rec, den)
                    res = finp.tile([P, NT, OUT_DIM], F32)
                    nc.vector.tensor_tensor(
                        res,
                        accs[:, :, 0:OUT_DIM],
                        rec[:, :, None].to_broadcast([P, NT, OUT_DIM]),
                        mybir.AluOpType.mult,
                    )
                    nc.sync.dma_start(out.rearrange("(t p) d -> p t d", p=P), res)

    nc.finalize()
    return nc


def make_in_maps(cfg, struct, data, h, W_fc, a_attn):
    NPC, NPAD, NTOT = cfg.NPC, cfg.NPAD, cfg.NTOT
    h = np.asarray(h, np.float32)
    W_fc = np.asarray(W_fc, np.float32)
    a_attn = np.asarray(a_attn, np.float32)

    w_s = W_fc.T @ a_attn[:OUT_DIM]     # [256]
    w_d = W_fc.T @ a_attn[OUT_DIM:]
    W_aug = np.concatenate(
        [W_fc.T, w_s[:, None], w_d[:, None]], axis=1
    ).astype(ml_dtypes.bfloat16)        # [256, 130]

    ident = np.eye(P, dtype=ml_dtypes.bfloat16)

    # global relabeled hT in bf16: col (c*NPAD + s) = h[c*NPC + order0[c][s]]
    order0 = struct["order0"]
    hT_rel = np.zeros((IN_DIM, NTOT), ml_dtypes.bfloat16)
    hbf = h.astype(ml_dtypes.bfloat16)
    for c in range(NC):
        o = order0[c]
        valid = o < NPC
        cols = np.zeros((NPAD, IN_DIM), ml_dtypes.bfloat16)
        cols[valid] = hbf[c * NPC + o[valid]]
        hT_rel[:, c * NPAD:(c + 1) * NPAD] = cols.T
    hT_rel = np.ascontiguousarray(hT_rel)

    in_maps = []
    for c in range(NC):
        d = data[c]
        in_maps.append({
            "hT": hT_rel,
            "hTown": np.ascontiguousarray(hT_rel[:, c * NPAD:(c + 1) * NPAD]),
            "W_aug": np.ascontiguousarray(W_aug),
            "ident": ident,
            "gidx0": np.ascontiguousarray(d["gidx0"]),
            "gmask0": np.ascontiguousarray(d["gmask0"]),
            "gidx1": np.ascontiguousarray(d["gidx1"]),
            "gmask1": np.ascontiguousarray(d["gmask1"]),
            "mscat1": np.ascontiguousarray(d["mscat1"]),
            "zdscat1": np.ascontiguousarray(d["zdscat1"]),
        })
    return in_maps


def run(h, src, dst, W_fc, a_attn, n_nodes=None, n_edges=None, trace=False):
    h = np.asarray(h, np.float32)
    cfg = Cfg(
        n_nodes if n_nodes is not None else h.shape[0],
        n_edges if n_edges is not None else np.asarray(src).shape[0],
    )
    struct, data = host_prep(cfg, src, dst)
    nc = build_program(cfg, struct)
    in_maps = make_in_maps(cfg, struct, data, h, W_fc, a_attn)
    results = run_bass_kernel_spmd(
        nc, in_maps, core_ids=list(range(NC)), trace=trace
    )
    # un-relabel: out row s of core c -> original node c*NPC + order0[c][s]
    order0 = struct["order0"]
    full = np.zeros((cfg.N, OUT_DIM), np.float32)
    for c, r in enumerate(results.results):
        o = order0[c]
        valid = o < cfg.NPC
        full[c * cfg.NPC + o[valid]] = r["out"][valid]
    return full, results


def kernel(h, src, dst, W_fc, a_attn):
    full, _ = run(h, src, dst, W_fc, a_attn)
    return full
